# revision 21
# baseline (speedup 1.0000x reference)
"""GATConv (PyG defaults: add_self_loops, concat=False/head-mean) on 8 Trainium2 cores.

Strategy: edges are bucketed by DESTINATION node. Core k owns the NPC-node
slice [k*NPC, (k+1)*NPC) and every edge whose dst lands there, so the
segment softmax and the message aggregation are entirely core-local — no
device collectives. The host concatenates the 8 output slices.

v2 design notes (why it is shaped this way):
  - exp(lrelu(z)) = max(exp(z), exp(0.2 z)) and z = a_s[src]+a_d[dst], so
    e = max(es1[src]*ed1[dst], es2[src]*ed2[dst]) with es/ed computed ONCE
    per node in phase 1 (ACT engine) and stored in the h-table row.
  - the per-edge a_d (dst side) is selected with a one-hot matmul
    psum_ad = s01T^T @ adblk on the PE instead of a per-edge dma_gather:
    Q7 descriptor generation (~7.8 ns/idx) was the previous bottleneck.
  - the one-hot matrices s01 [e,d] / s01T [d,e] are precomputed on the host
    and streamed as f16, removing all is_equal work from the DVE.
  - per-edge h rows (1280 B) are gathered with dma_gather; pad slots use
    idx -1 ("negative indices at the end are ignored") and the es slots of
    each gather buffer are zero-filled first so stale data can't poison
    the pipeline (everything downstream multiplies by an exact 0).

Device program (SPMD-identical across cores; all per-core structure lives
in host-supplied index arrays):

  Phase 1 (replicated): one pass over x: h = x @ W (PE) -> f16, plus
      es1,es2,ed1,ed2 = exp-variants of x@w_as, x@w_ad (ACT) -> fp16
      h-table [NPAD, 640] rows: [h 512 | es 8 | ed 8 | pad], 1280 B.

  Phase 2, per dst block b (128 dsts):
    - INDIRECT1D: selfad[128, 8] = ed slots of the block's own nodes
    - per (block, half): dma_gather of t_half*128 h rows (halved so the
      int16 gather indices cover the table); s01/s01T streamed by dma
    - per 128-edge tile: psum_ad = s01T^T @ selfad (PE); e1 = es1*ed1,
      e2 = es2*ed2, ev = max(e1,e2) -> f16 (DVE); hw[:,hd*128:] =
      h_src*ev[hd] (3 on DVE, 1 on ACT); psum_out += s01^T @ hw,
      psum_den += s01^T @ ev (PE)
    - block end: out = sum_h psum_out[:,h]/(H*den[:,h]) -> HBM
"""

import math
import sys

import numpy as np

if "/opt/trn_rl_repo" not in sys.path:
    sys.path.insert(0, "/opt/trn_rl_repo")

P = 128
SLOPE = 0.2
LN16 = float(np.log(16.0))
HROW = 640          # padded h-table row (f16 elems): 512 h + 8 es + 8 ed + pad
HUSE = 528          # used columns of the h row


class Cfg:
    def __init__(self, N=50000, E=800000, DIN=128, DOUT=128, H=4, ncores=8):
        self.N, self.E, self.DIN, self.DOUT, self.H = N, E, DIN, DOUT, H
        self.NCORES = ncores
        self.NPC = N // ncores                 # nodes per core
        self.NBLK = math.ceil(self.NPC / P)    # dst blocks per core
        self.LAST_ROWS = self.NPC - (self.NBLK - 1) * P
        self.NPAD = math.ceil(N / P) * P       # padded node count
        self.NTILE_N = self.NPAD // P          # node tiles in phase 1
        self.WH = H * DOUT                     # h width = 512
        self.NSPLIT = self.NPAD // 2           # h-table half split row
        assert DIN == P and self.WH == 512 and H * DOUT == 512
        assert self.NSPLIT < 32768 and self.NPAD - self.NSPLIT < 32768


DEFAULT_CFG = Cfg()


def _build_program(cfg: Cfg, t_half: int):
    """nt = NBLK * 2 * t_half edge tiles per core, all data via index arrays."""
    from contextlib import ExitStack

    import concourse.bacc as bacc
    import concourse.bass as bass
    import concourse.mybir as mybir
    import concourse.tile as tile

    f16 = mybir.dt.float16
    f32 = mybir.dt.float32
    i32 = mybir.dt.int32
    i16 = mybir.dt.int16
    AF = mybir.ActivationFunctionType
    ALU = mybir.AluOpType
    WH, H, DOUT = cfg.WH, cfg.H, cfg.DOUT
    nt = cfg.NBLK * 2 * t_half
    NIH = t_half * P          # idxs per h-gather group (one (block, half))

    nc = bacc.Bacc(
        "TRN2",
        target_bir_lowering=False,
        debug=False,
        enable_asserts=False,
        num_devices=cfg.NCORES,
    )

    xT = nc.dram_tensor("xT", [P, cfg.NPAD], f16, kind="ExternalInput").ap()
    wext = nc.dram_tensor("wext", [P, WH + 2 * H], f16, kind="ExternalInput").ap()
    hidx_in = nc.dram_tensor("hidx", [P, nt * 8], i16, kind="ExternalInput").ap()
    s01_in = nc.dram_tensor("s01", [P, nt * P], f16, kind="ExternalInput").ap()
    s01T_in = nc.dram_tensor("s01T", [P, nt * P], f16, kind="ExternalInput").ap()
    bidx_in = nc.dram_tensor("bidx", [P, cfg.NBLK], i32, kind="ExternalInput").ap()
    out = nc.dram_tensor("out", [cfg.NPC, DOUT], f32, kind="ExternalOutput").ap()
    htab = nc.dram_tensor("htab", [cfg.NPAD, HROW], f16, kind="Internal").ap()

    with tile.TileContext(nc) as tc:
        with ExitStack() as ctx:
            cpool = ctx.enter_context(tc.tile_pool(name="const", bufs=1))
            wext_sb = cpool.tile([P, WH + 2 * H], f16)
            nc.sync.dma_start(wext_sb[:], wext[:, :])
            bidx_sb = cpool.tile([P, cfg.NBLK], i32)
            nc.sync.dma_start(bidx_sb[:], bidx_in[:, :])
            bln16 = cpool.tile([P, 1], f32)
            nc.vector.memset(bln16[:], -LN16)

            # ---------------- Phase 1: h table ----------------
            XB = 4  # node tiles per x DMA
            with (
                tc.tile_pool(name="p1x", bufs=3) as p1x,
                tc.tile_pool(name="p1h", bufs=4) as p1h,
                tc.tile_pool(name="p1ph", bufs=4, space="PSUM") as p1ph,
                tc.tile_pool(name="p1ps", bufs=4, space="PSUM") as p1ps,
            ):
                for t4 in range(math.ceil(cfg.NTILE_N / XB)):
                    nb = min(XB, cfg.NTILE_N - t4 * XB)
                    xt = p1x.tile([P, XB * P], f16)
                    nc.sync.dma_start(
                        xt[:, 0 : nb * P],
                        xT[:, t4 * XB * P : t4 * XB * P + nb * P],
                    )
                    for j in range(nb):
                        t = t4 * XB + j
                        xj = xt[:, j * P : (j + 1) * P]
                        ph = p1ph.tile([P, WH], f32, space="PSUM")
                        ps = p1ps.tile([P, 2 * H], f32, space="PSUM")
                        nc.tensor.matmul(
                            ph[:], lhsT=xj, rhs=wext_sb[:, 0:WH],
                            start=True, stop=True,
                        )
                        nc.tensor.matmul(
                            ps[:], lhsT=xj, rhs=wext_sb[:, WH:],
                            start=True, stop=True,
                        )
                        hsb = p1h.tile([P, HUSE], f16)
                        c0 = (WH * 3) // 4
                        nc.vector.tensor_copy(hsb[:, 0:c0], ph[:, 0:c0])
                        nc.scalar.activation(hsb[:, c0:WH], ph[:, c0:WH], AF.Copy)
                        # slots [es1 ed1 | es2 ed2]; the common 1/16 in both
                        # exp groups cancels in the softmax, it only guards
                        # the f16 range of the products
                        nc.scalar.activation(
                            hsb[:, WH : WH + 2 * H], ps[:, 0 : 2 * H], AF.Exp,
                            bias=bln16[:],
                        )
                        nc.scalar.activation(
                            hsb[:, WH + 2 * H : WH + 4 * H], ps[:, 0 : 2 * H],
                            AF.Exp, bias=bln16[:], scale=SLOPE,
                        )
                        nc.sync.dma_start(
                            htab[t * P : (t + 1) * P, 0:HUSE], hsb[:]
                        )

            tc.strict_bb_all_engine_barrier()

            # ---------------- Phase 2a: per-block ed table ----------------
            # gather cols [516:528) = [ed1 | es2 | ed2] of each block's own
            # 128 dst nodes; barrier before use (indirect-DMA completion)
            adall = cpool.tile([P, cfg.NBLK * 3 * H], f16)
            for b in range(cfg.NBLK):
                nc.gpsimd.indirect_dma_start(
                    out=adall[:, b * 3 * H : (b + 1) * 3 * H],
                    out_offset=None,
                    in_=htab[:, :],
                    in_offset=bass.IndirectOffsetOnAxis(
                        ap=bidx_sb[:, b : b + 1], axis=0
                    ),
                    element_offset=WH + H,
                )

            tc.strict_bb_all_engine_barrier()

            # ---------------- Phase 2b: edge processing ----------------
            hidx = cpool.tile([P, nt * 8], i16)
            nc.sync.dma_start(hidx[:], hidx_in[:, :])

            htabA = htab[0 : cfg.NSPLIT, :]
            htabB = htab[cfg.NSPLIT :, :]

            with (
                tc.tile_pool(name="gh", bufs=3) as gh_pool,
                tc.tile_pool(name="s01p", bufs=3) as s01_pool,
                tc.tile_pool(name="s01tp", bufs=3) as s01t_pool,
                tc.tile_pool(name="hwp", bufs=4) as hw_pool,
                tc.tile_pool(name="sm", bufs=8) as sm_pool,
                tc.tile_pool(name="evp", bufs=6) as ev_pool,
                tc.tile_pool(name="fin", bufs=3) as fin_pool,
                tc.tile_pool(name="pso", bufs=2, space="PSUM") as pso_pool,
                tc.tile_pool(name="psd", bufs=2, space="PSUM") as psd_pool,
                tc.tile_pool(name="psa", bufs=3, space="PSUM") as psa_pool,
            ):
                for b in range(cfg.NBLK):
                    sad = adall[:, b * 3 * H : (b + 1) * 3 * H]
                    psum_out = pso_pool.tile([P, WH], f32, space="PSUM")
                    psum_den = psd_pool.tile([P, H], f32, space="PSUM")
                    for hh in range(2):
                        g = b * 2 + hh  # h-gather group id
                        gh = gh_pool.tile([P, t_half * HROW], f16)
                        nc.gpsimd.dma_gather(
                            out_ap=gh[:].rearrange("p (k e) -> p k e", e=HROW),
                            in_ap=htabA if hh == 0 else htabB,
                            idxs_ap=hidx[:, g * NIH // 16 : (g + 1) * NIH // 16],
                            num_idxs=NIH,
                            num_idxs_reg=NIH,
                            elem_size=HROW,
                            single_packet=False,
                        )
                        s01g = s01_pool.tile([P, t_half * P], f16)
                        nc.sync.dma_start(
                            s01g[:], s01_in[:, g * NIH : (g + 1) * NIH]
                        )
                        s01tg = s01t_pool.tile([P, t_half * P], f16)
                        nc.sync.dma_start(
                            s01tg[:], s01T_in[:, g * NIH : (g + 1) * NIH]
                        )
                        for s in range(t_half):
                            tt = hh * t_half + s  # slot in block order
                            gj = gh[:, s * HROW : s * HROW + WH]
                            s01 = s01g[:, s * P : (s + 1) * P]
                            s01t = s01tg[:, s * P : (s + 1) * P]
                            # per-edge ed via one-hot select on the PE
                            pad = psa_pool.tile([P, 3 * H], f32, space="PSUM")
                            nc.tensor.matmul(
                                pad[:], lhsT=s01t, rhs=sad,
                                start=True, stop=True,
                            )
                            esb = sm_pool.tile([P, 4 * H], f32, tag="esb")
                            nc.vector.tensor_copy(
                                esb[:],
                                gh[:, s * HROW + WH : s * HROW + WH + 4 * H],
                            )
                            e1 = sm_pool.tile([P, H], f32, tag="e1")
                            nc.vector.tensor_tensor(
                                out=e1[:], in0=esb[:, 0:H], in1=pad[:, 0:H],
                                op=ALU.mult,
                            )
                            e2 = sm_pool.tile([P, H], f32, tag="e2")
                            nc.vector.tensor_tensor(
                                out=e2[:],
                                in0=esb[:, 2 * H : 3 * H],
                                in1=pad[:, 2 * H : 3 * H],
                                op=ALU.mult,
                            )
                            ev16 = ev_pool.tile([P, H], f16)
                            nc.vector.tensor_tensor(
                                out=ev16[:], in0=e1[:], in1=e2[:], op=ALU.max
                            )
                            hw = hw_pool.tile([P, WH], f16)
                            for hd in range(H):
                                nc.vector.tensor_tensor(
                                    out=hw[:, hd * DOUT : (hd + 1) * DOUT],
                                    in0=ev16[:, hd : hd + 1].to_broadcast(
                                        [P, DOUT]
                                    ),
                                    in1=gj[:, hd * DOUT : (hd + 1) * DOUT],
                                    op=ALU.mult,
                                )
                            first = tt == 0
                            last = tt == 2 * t_half - 1
                            nc.tensor.matmul(
                                psum_out[:], lhsT=s01, rhs=hw[:],
                                start=first, stop=last,
                            )
                            nc.tensor.matmul(
                                psum_den[:], lhsT=s01, rhs=ev16[:],
                                start=first, stop=last,
                            )
                    # normalize + head mean
                    den4 = sm_pool.tile([P, H], f32, tag="den4")
                    nc.vector.tensor_scalar_mul(den4[:], psum_den[:], float(H))
                    rec = sm_pool.tile([P, H], f32, tag="rec")
                    nc.vector.reciprocal(rec[:], den4[:])
                    acc = fin_pool.tile([P, DOUT], f32)
                    nc.vector.tensor_scalar_mul(
                        acc[:], psum_out[:, 0:DOUT], rec[:, 0:1]
                    )
                    for hd in range(1, H):
                        nc.vector.scalar_tensor_tensor(
                            out=acc[:],
                            in0=psum_out[:, hd * DOUT : (hd + 1) * DOUT],
                            scalar=rec[:, hd : hd + 1],
                            in1=acc[:],
                            op0=ALU.mult,
                            op1=ALU.add,
                        )
                    rows = cfg.LAST_ROWS if b == cfg.NBLK - 1 else P
                    nc.sync.dma_start(
                        out[b * P : b * P + rows, :], acc[:rows, :]
                    )

    nc.compile()
    return nc


def _wrap16(idx_flat, ni_per_group):
    """[G*NI] edge-position-ordered idx -> [128, G*NI/16] wrapped-16 layout,
    replicated across the 8 16-partition groups."""
    g = idx_flat.reshape(-1, ni_per_group)
    ng = g.shape[0]
    w = np.zeros((16, ng, ni_per_group // 16), np.int16)
    for p in range(16):
        w[p] = g[:, p::16]
    w = w.reshape(16, ng * (ni_per_group // 16))
    return np.tile(w, (8, 1))


def _prep(cfg: Cfg, x, edge_index, W, att_src, att_dst):
    """Host-side sharding/preprocessing -> (per-core in_maps, t_half)."""
    f16 = np.float16
    N, H, DIN, DOUT = cfg.N, cfg.H, cfg.DIN, cfg.DOUT
    src = np.concatenate([np.asarray(edge_index[0]), np.arange(N)]).astype(np.int64)
    dst = np.concatenate([np.asarray(edge_index[1]), np.arange(N)]).astype(np.int64)

    # sort edges by (core, block, src-half): key = dst-block * 2 + half
    core = dst // cfg.NPC
    ln = dst - core * cfg.NPC
    half = (src >= cfg.NSPLIT).astype(np.int64)
    key = (core * cfg.NBLK + ln // P) * 2 + half
    order = np.argsort(key, kind="stable")
    src_s = src[order].astype(np.int32)
    ln_s = ln[order].astype(np.int32)
    key_s = key[order]

    nseg = cfg.NCORES * cfg.NBLK * 2
    counts = np.bincount(key_s, minlength=nseg)
    t_half = int(max(1, ((counts + P - 1) // P).max()))
    seg_len = t_half * P
    nt = cfg.NBLK * 2 * t_half

    starts = np.concatenate([[0], np.cumsum(counts)])
    # padded per-core flat arrays in (block, half, slot, partition) order.
    # pad h-gather slots use idx 0 (a real row, so gathered data stays
    # finite); pad correctness comes from the zero s01/s01T columns.
    hsrc = np.zeros((cfg.NCORES, nt * P), np.int32)     # half-rebased src
    dl_i = np.full((cfg.NCORES, nt * P), -1, np.int32)  # local dst (-1 = pad)
    for c in range(cfg.NCORES):
        for b in range(cfg.NBLK):
            for hh in range(2):
                seg = (c * cfg.NBLK + b) * 2 + hh
                s, e = starts[seg], starts[seg + 1]
                cnt = e - s
                o = ((b * 2 + hh) * t_half) * P
                sr = src_s[s:e] - (cfg.NSPLIT if hh else 0)
                hsrc[c, o : o + cnt] = sr
                dl_i[c, o : o + cnt] = ln_s[s:e] - b * P

    # one-hot matrices: s01[lane, tile, dstcol], s01T[dstrow, tile, lanecol]
    lanes = np.tile(np.arange(P, dtype=np.int64), nt)
    tiles = np.repeat(np.arange(nt, dtype=np.int64), P)
    s01_all = []
    s01T_all = []
    for c in range(cfg.NCORES):
        ld = dl_i[c]
        m = ld >= 0
        s01 = np.zeros((P, nt, P), f16)
        s01[lanes[m], tiles[m], ld[m]] = 1
        s01T = np.zeros((P, nt, P), f16)
        s01T[ld[m], tiles[m], lanes[m]] = 1
        s01_all.append(s01.reshape(P, nt * P))
        s01T_all.append(s01T.reshape(P, nt * P))

    xpad = np.zeros((cfg.NPAD, DIN), np.float32)
    xpad[:N] = np.asarray(x)
    xT = np.ascontiguousarray(xpad.T).astype(f16)
    Wn = np.asarray(W, dtype=np.float32)
    ws = np.einsum("khc,hc->kh", Wn.reshape(DIN, H, DOUT), np.asarray(att_src, np.float32))
    wd = np.einsum("khc,hc->kh", Wn.reshape(DIN, H, DOUT), np.asarray(att_dst, np.float32))
    wext = np.concatenate([Wn, ws, wd], axis=1).astype(f16)
    # block node ids (global), clamped to the core's range
    in_maps = []
    for c in range(cfg.NCORES):
        bid = (
            c * cfg.NPC
            + np.minimum(
                np.arange(cfg.NBLK)[None, :] * P + np.arange(P)[:, None],
                cfg.NPC - 1,
            )
        ).astype(np.int32)
        in_maps.append(
            {
                "xT": xT,
                "wext": wext,
                "hidx": _wrap16(hsrc[c], t_half * P),
                "s01": s01_all[c],
                "s01T": s01T_all[c],
                "bidx": bid,
            }
        )
    return in_maps, t_half


def run(cfg: Cfg, x, edge_index, W, att_src, att_dst, trace=False, sim=False):
    in_maps, t_half = _prep(cfg, x, edge_index, W, att_src, att_dst)
    nc = _build_program(cfg, t_half)
    if sim:
        from concourse.bass_interp import CoreSim

        outs = []
        for c in range(cfg.NCORES):
            s = CoreSim(nc, trace=False, require_finite=False, require_nnan=False)
            for k, v in in_maps[c].items():
                s.tensor(k)[:] = v
            s.simulate(check_with_hw=False)
            outs.append(np.array(s.tensor("out")))
        return np.concatenate(outs, axis=0), None
    from concourse.bass_utils import run_bass_kernel_spmd

    res = run_bass_kernel_spmd(
        nc, in_maps, core_ids=list(range(cfg.NCORES)), trace=trace
    )
    out = np.concatenate([r["out"] for r in res.results], axis=0)
    return out.astype(np.float32), res


def kernel(x, edge_index, W, att_src, att_dst):
    out, _ = run(DEFAULT_CFG, x, edge_index, W, att_src, att_dst)
    return out
